# revision 34
# baseline (speedup 1.0000x reference)
"""Trainium2 Bass kernel for nn_MultiHeadFast (multi-head attention with
softmax over the QUERY axis).

Math (faithful to the reference):
  qkv = x @ Ws;  per (b,h):  S[q,k] = Q.K^T,  causal mask k<=q,
  P = softmax_over_q(S * T^-0.5),  out = P @ V.

v2 design (PE-minimal):
  - Sharding: 8 cores = 2 batches x 4 head-groups.  Core c owns batch c//4
    and 4 heads, processed as 2 passes of 2 heads (PSUM limit).
  - Host passes x^T (bf16) and the per-core Ws column slice (bf16), so the
    device does ZERO input transposes.  V is produced token-major via an
    XBAR DMA transpose (out[p,i,d] = in[d, i*128+p]), not the PE.
  - S^T is computed only on the live causal region (q >= 128*ktile) in
    <=512-col matmuls; exp (query-axis softmax numerator) on ScalarE;
    per-key normalizers via VectorE free-axis reduce of the bf16 strip.
  - out^T[d,q] = sum_k V_norm^T P^T accumulates in PSUM and is DMA'd out
    transposed; the host does the final cheap (128,2048)->(2048,128)
    transpose.  Pipeline: [QKV pass0] [S/exp pass0 + QKV pass1 on PE]
    [S/exp pass1 + PV pass0 + PV pass1 slab-major] [PV1 tail].
"""

import numpy as np
import ml_dtypes
from contextlib import ExitStack

import concourse.bass as bass
import concourse.mybir as mybir
import concourse.tile as tile
from concourse import bacc
from concourse.bass_utils import run_bass_kernel_spmd

B, T, E = 2, 2048, 1024
H, D = 16, 64
NCORES = 8
P = 128
EK = E // P           # 8 contraction blocks
KT = T // P           # 16 key tiles per core-batch
NS = T // 512         # 4 query slabs
DT = mybir.dt.bfloat16
F32 = mybir.dt.float32
SCALE = float(T) ** -0.5
NEG = -1e30


def live(k):
    return T - P * k


def build_kernel():
    nc = bacc.Bacc("TRN2", target_bir_lowering=False, debug=False)
    x_dram = nc.dram_tensor("x", (E, T), DT, kind="ExternalInput")       # x^T
    w_dram = nc.dram_tensor("wsl", (E, 768), DT, kind="ExternalInput")
    out_dram = nc.dram_tensor("out", (2, P, T), F32, kind="ExternalOutput")

    with tile.TileContext(nc) as tc, ExitStack() as ctx:
        const = ctx.enter_context(tc.tile_pool(name="const", bufs=1))
        xp = ctx.enter_context(tc.tile_pool(name="xp", bufs=1))
        qkvp = ctx.enter_context(tc.tile_pool(name="qkvp", bufs=1))
        strips = ctx.enter_context(tc.tile_pool(name="strips", bufs=1))
        small = ctx.enter_context(tc.tile_pool(name="small", bufs=1))
        ps = ctx.enter_context(tc.tile_pool(name="ps", bufs=1, space="PSUM"))

        # ---- constants ----
        zeros_bf = const.tile([P, P], DT, name="zeros_bf")
        nc.gpsimd.memset(zeros_bf[:], 0.0)
        # diagmask[p, f] = 0 if f >= p else NEG (keys on partitions, q free)
        diagmask = const.tile([P, P], F32, name="diagmask")
        nc.gpsimd.memset(diagmask[:], 0.0)
        nc.gpsimd.affine_select(
            out=diagmask[:],
            in_=diagmask[:],
            compare_op=mybir.AluOpType.is_ge,
            fill=NEG,
            base=0,
            pattern=[[1, P]],
            channel_multiplier=-1,
        )

        # ---- PE warm-up: ~4us of dummy matmuls during the input DMA wait
        # so the HAM clock-gate opens (1.2 -> 2.4 GHz) before real work ----
        warm = const.tile([P, 512], DT, name="warm")
        nc.gpsimd.memset(warm[:], 0.0)
        wps = ps.tile([P, 512], F32, tag="b512", bufs=4, name="warm_ps")
        for _ in range(10):
            nc.tensor.matmul(wps[:], lhsT=zeros_bf[:], rhs=warm[:],
                             start=True, stop=True, skip_group_check=True)

        # ---- input DMAs (two HWDGE queues; slab 3 + Q/K weights first) ----
        wsl = qkvp.tile([P, EK, 768], DT, name="wsl")
        wr = w_dram.rearrange("(eo ei) f -> ei eo f", ei=P)
        xT = xp.tile([P, EK, T], DT, name="xT")

        def xslab_dma(eng, s):
            eng.dma_start(
                xT[:, :, 512 * s : 512 * (s + 1)],
                x_dram[:, 512 * s : 512 * (s + 1)].rearrange(
                    "(eo ei) t -> ei eo t", ei=P
                ),
            )

        nc.sync.dma_start(wsl[:, :, 0:384], wr[:, :, 0:384])
        xslab_dma(nc.sync, 3)
        xslab_dma(nc.scalar, 0)
        xslab_dma(nc.sync, 1)
        xslab_dma(nc.scalar, 2)
        nc.sync.dma_start(wsl[:, :, 384:768], wr[:, :, 384:768])

        # ---- per-pass tensors ----
        qt = [qkvp.tile([P, T], DT, name=f"qt{p}") for p in range(2)]
        kt = [qkvp.tile([P, T], DT, name=f"kt{p}") for p in range(2)]
        vt = [qkvp.tile([P, T], DT, name=f"vt{p}") for p in range(2)]
        vnat = [qkvp.tile([P, KT, P], DT, name=f"vn{p}") for p in range(2)]
        vp_all = [qkvp.tile([P, KT, 2, D], DT, name=f"vp{p}") for p in range(2)]
        rsum_all = [qkvp.tile([P, 2 * KT], F32, name=f"rs{p}") for p in range(2)]

        def qkv_unit(p, m, s):
            """One 512-token slab of Q^T/K^T/V^T (m=0/1/2) for pass p.
            Q and K run as fp8 DoubleRow (contraction 256 per matmul);
            V stays bf16 for accuracy (its error passes straight through)."""
            dst = (qt, kt, vt)[m][p]
            mm = ps.tile([P, 512], F32, tag="b512", bufs=4, name="qkv_ps")
            for e in range(EK):
                nc.tensor.matmul(
                    mm[:],
                    lhsT=wsl[:, e, 256 * m + P * p : 256 * m + P * (p + 1)],
                    rhs=xT[:, e, 512 * s : 512 * (s + 1)],
                    start=(e == 0),
                    stop=(e == EK - 1),
                )
            nc.vector.tensor_copy(dst[:, 512 * s : 512 * (s + 1)], mm[:])
            if m == 2:
                # V natural layout for this slab: vnat[p_, i, d] = vt[d, i*128+p_]
                nc.sync.dma_start_transpose(
                    vnat[p][:, 4 * s : 4 * (s + 1), :],
                    vt[p][:, 512 * s : 512 * (s + 1)],
                )

        def s_exp_pair(p, k, strips_kh):
            """S^T matmuls + mask + exp + normalizer for both heads of a
            (pass, ktile).  Head h uses PE row-groups h*64..h*64+63, so
            interleaving heads lets LDWEIGHTS overlap in-flight matmuls."""
            L = live(k)
            q0 = P * k
            parts = {0: [], 1: []}
            for c in range(0, L, 1024):
                cw = min(1024, L - c)
                sps = {}
                for h in range(2):
                    sps[h] = ps.tile([P, 1024], F32, tag="sps", bufs=2, name="sps")
                for so in range(0, cw, 512):
                    w = min(512, cw - so)
                    for h in range(2):
                        nc.tensor.matmul(
                            sps[h][:, so : so + w],
                            lhsT=kt[p][h * D : (h + 1) * D, q0 : q0 + P],
                            rhs=qt[p][h * D : (h + 1) * D,
                                      q0 + c + so : q0 + c + so + w],
                            start=True,
                            stop=True,
                        )
                for h in range(2):
                    if c == 0:
                        nc.vector.tensor_add(sps[h][:, 0:P], sps[h][:, 0:P], diagmask[:])
                    acc = small.tile([P, 1], F32, tag="acc", bufs=8, name="acc")
                    nc.scalar.activation(
                        strips_kh[h][:, c : c + cw],
                        sps[h][:, :cw],
                        mybir.ActivationFunctionType.Exp,
                        scale=SCALE,
                        accum_out=acc[:],
                    )
                    parts[h].append(acc)
            for h in range(2):
                if len(parts[h]) == 1:
                    ssum = parts[h][0]
                else:
                    ssum = small.tile([P, 1], F32, tag="acc", bufs=8, name="ssum")
                    nc.vector.tensor_add(ssum[:], parts[h][0][:], parts[h][1][:])
                nc.vector.reciprocal(
                    rsum_all[p][:, 2 * k + h : 2 * k + h + 1], ssum[:]
                )

        def s_exp_tailpair(p, k, st_a, st_b):
            """Two small k-tiles (live<=512 each) share one PSUM allocation
            per head, halving the S->exp dependency round-trips in the tail."""
            la, lb = live(k), live(k + 1)
            sps = {}
            for h in range(2):
                sps[h] = ps.tile([P, 1024], F32, tag="sps", bufs=2, name="spst")
            for h in range(2):
                for kk, off, ll in ((k, 0, la), (k + 1, 512, lb)):
                    nc.tensor.matmul(
                        sps[h][:, off : off + ll],
                        lhsT=kt[p][h * D : (h + 1) * D, P * kk : P * kk + P],
                        rhs=qt[p][h * D : (h + 1) * D, P * kk : P * kk + ll],
                        start=True,
                        stop=True,
                    )
            for h in range(2):
                for (kk, off, ll, st) in ((k, 0, la, st_a), (k + 1, 512, lb, st_b)):
                    nc.vector.tensor_add(
                        sps[h][:, off : off + P], sps[h][:, off : off + P], diagmask[:]
                    )
                    acc = small.tile([P, 1], F32, tag="acc", bufs=8, name="acc")
                    nc.scalar.activation(
                        st[h][:, 0:ll],
                        sps[h][:, off : off + ll],
                        mybir.ActivationFunctionType.Exp,
                        scale=SCALE,
                        accum_out=acc[:],
                    )
                    nc.vector.reciprocal(
                        rsum_all[p][:, 2 * kk + h : 2 * kk + h + 1], acc[:]
                    )

        def pv_mms(p, k, h, strip, pv, j, last):
            """PV contribution of (pass, ktile, head) to out^T slab j."""
            j0 = k // 4
            if j == j0:
                coff = P * (k % 4)
                nc.tensor.matmul(
                    pv[h * D : (h + 1) * D, coff:512],
                    lhsT=vp_all[p][:, k, h, :],
                    rhs=strip[:, 0 : 512 - coff],
                    start=False,
                    stop=last,
                    skip_group_check=True,
                )
            else:
                c = 512 * j - P * k
                nc.tensor.matmul(
                    pv[h * D : (h + 1) * D, :],
                    lhsT=vp_all[p][:, k, h, :],
                    rhs=strip[:, c : c + 512],
                    start=False,
                    stop=last,
                    skip_group_check=True,
                )

        def dma_out_slab(p, j, pv):
            ob = strips.tile([P, 512], F32, tag="outb", bufs=2, name="outb")
            nc.vector.tensor_copy(ob[:], pv[:])
            nc.sync.dma_start(out_dram[p, :, 512 * j : 512 * (j + 1)], ob[:])

        def new_pv_bank():
            pv = ps.tile([P, 512], F32, tag="b512", bufs=4, name="pv")
            nc.tensor.matmul(
                pv[:],
                lhsT=zeros_bf[:],
                rhs=xT[:, 0, 0:512],
                start=True,
                stop=False,
                skip_group_check=True,
            )
            return pv

        stripd = [{}, {}]
        pvd = {}

        def vp_scale(p, k0, k1):
            for kk in range(k0, k1):
                for h in range(2):
                    nc.vector.tensor_scalar_mul(
                        vp_all[p][:, kk, h, :],
                        vnat[p][:, kk, :][:, h * D : (h + 1) * D],
                        rsum_all[p][:, 2 * kk + h : 2 * kk + h + 1],
                    )

        def pv_open(p, j):
            pvd[(p, j)] = new_pv_bank()

        def pv_add(p, j, k0, k1, last=False):
            """Contributions of k-tiles [k0, k1) to out^T slab j; `last`
            closes the accumulation group on the final matmul."""
            for kk in range(k0, k1):
                for h in range(2):
                    pv_mms(p, kk, h, stripd[p][(kk, h)], pvd[(p, j)], j,
                           last=(last and kk == k1 - 1 and h == 1))
            if last:
                dma_out_slab(p, j, pvd[(p, j)])

        # PE filler units (QKV slabs) spread through the exp stream so the
        # tensor engine never idles long enough for the HAM clock-gate to
        # re-throttle.  Each pass runs its k-tiles in order [12..15, 0..11]:
        # the small tail tiles only need QKV slab 3, so exp starts ~7us in.
        PAIRF = {
            0: [[(0, 0, 0), (0, 0, 1)], [(0, 0, 2), (0, 1, 0)]],
            1: [[(1, 0, 0), (1, 0, 1)], [(1, 0, 2), (1, 1, 0)]],
        }
        KFILL = {
            0: {1: (0, 2, 0), 2: (0, 1, 1), 3: (0, 2, 1), 5: (0, 1, 2),
                6: (0, 2, 3), 7: (1, 0, 3), 9: (0, 2, 2), 10: (1, 1, 3)},
            1: {1: (1, 2, 0), 2: (1, 1, 1), 3: (1, 2, 1), 5: (1, 1, 2),
                6: (1, 2, 3), 9: (1, 2, 2)},
        }

        # prelude: what the pass-0 tail pairs need (QKV slab 3)
        qkv_unit(0, 0, 3)
        qkv_unit(0, 1, 3)

        def new_strips(p, k):
            sts = [
                strips.tile([P, live(k)], DT, tag=f"s{k}", bufs=2,
                            name=f"s{p}_{k}")
                for _ in range(2)
            ]
            for h in range(2):
                stripd[p][(k, h)] = sts[h]
            return sts

        for p in range(2):
            for i, kk0 in enumerate((12, 14)):
                s_exp_tailpair(p, kk0, new_strips(p, kk0), new_strips(p, kk0 + 1))
                for u in PAIRF[p][i]:
                    qkv_unit(*u)
            for k in range(12):
                s_exp_pair(p, k, new_strips(p, k))
                if k in KFILL[p]:
                    qkv_unit(*KFILL[p][k])
                elif k >= 4:
                    # keep-warm: accumulate +0 into an open pv bank so the
                    # HAM clock-gate never re-throttles in fill-less spots
                    jw = 3 if k > 8 else (k - 1) // 4
                    for _ in range(3):
                        nc.tensor.matmul(
                            pvd[(p, jw)][:], lhsT=zeros_bf[:], rhs=warm[:],
                            start=False, stop=False, skip_group_check=True,
                        )
                if k == 0:
                    pv_open(p, 0)
                elif k == 4:
                    vp_scale(p, 0, 4)
                    pv_add(p, 0, 0, 4, last=True)
                    pv_open(p, 1)
                    pv_add(p, 1, 0, 4)
                elif k == 8:
                    vp_scale(p, 4, 8)
                    pv_add(p, 1, 4, 8, last=True)
                    pv_open(p, 2)
                    pv_add(p, 2, 0, 4)
                    pv_open(p, 3)
                    vp_scale(p, 12, 16)
                    pv_add(p, 3, 12, 16)
                elif k == 9:
                    pv_add(p, 2, 4, 8)
                    pv_add(p, 3, 0, 4)
                elif k == 10:
                    pv_add(p, 3, 4, 8)
            vp_scale(p, 8, 12)
            pv_add(p, 2, 8, 12, last=True)
            pv_add(p, 3, 8, 12, last=True)

    nc.compile()
    return nc


def prep_in_maps(x, Ws):
    x = np.asarray(x, np.float32)
    Ws = np.asarray(Ws, np.float32)
    in_maps = []
    for c in range(NCORES):
        b = c // 4
        xT = np.ascontiguousarray(x[b].T).astype(ml_dtypes.bfloat16)
        blocks = []
        for m in range(3):
            for p in range(2):
                g = (c % 4) * 4 + 2 * p
                blocks.append(Ws[:, m * E + D * g : m * E + D * g + 2 * D])
        wsl = np.concatenate(blocks, axis=1).astype(ml_dtypes.bfloat16)
        in_maps.append({"x": xT, "wsl": np.ascontiguousarray(wsl)})
    return in_maps


def assemble(results):
    out = np.empty((B, T, H * D), np.float32)
    for c in range(NCORES):
        r = np.asarray(results[c]["out"], np.float32)
        b = c // 4
        for p in range(2):
            for h in range(2):
                g = (c % 4) * 4 + 2 * p + h
                out[b, :, D * g : D * (g + 1)] = r[p, D * h : D * (h + 1), :].T
    return out


_NC_CACHE = None


def kernel(x: np.ndarray, Ws: np.ndarray) -> np.ndarray:
    global _NC_CACHE
    if _NC_CACHE is None:
        _NC_CACHE = build_kernel()
    nc = _NC_CACHE
    res = run_bass_kernel_spmd(nc, prep_in_maps(x, Ws), core_ids=list(range(NCORES)))
    return assemble(res.results)
